# revision 37
# baseline (speedup 1.0000x reference)
"""RGCN (basis-decomposition, one-hot features) message passing on 8 trn2 NeuronCores.

Math (per reference):
    weight[r] = sum_b comp[r,b] * basis[b]          # [R, N, OUT]
    msg_e     = weight[edge_type_e, src_e]          # [E, OUT]
    agg       = segment_sum(msg, dst) / max(cnt, 1) # mean aggregation
    out       = log_softmax(relu(agg + root + bias), axis=1)

Distribution strategy: shard edges by DESTINATION node range across the 8
cores (each core owns N/8 output rows), so no collectives are needed.

Per 128-edge block the device:
  1. gathers, per edge, the 5 basis rows of the src node from an fp8e4
     table (basisQ[n] = 32 * basis[:, n, :], 1536 B contiguous per node)
     via gpsimd dma_gather,
  2. accumulates psum[tile] += lhsT_b.T @ basisrows_b for b = 0..4, where
     lhsT_b[e, nd] = comp[type_e, b] * (dst_e == nd) is a host-prepared
     comp-scaled one-hot slab (pure scatter of comp values driven by the
     edge structure; no arithmetic), streamed sequentially from DRAM in
     fp8e4.  This keeps DVE free: no on-device one-hot construction.

The Q7 software descriptor generation of dma_gather has a large fixed
cost per call, so per-cell remainder blocks ("tails", <=128 rows) are
packed 16-aligned into per-half batches of 8 tails and fetched with ONE
gather call per batch, prefetched one 8-tile window ahead of their first
consumer; a tail that lands across a 128-slot column boundary simply
gets two tile-restricted lhsT slabs.  The epilogue (mean incl. the 1/32
dequant, +root, +bias, relu, log_softmax) runs fused per tile on
DVE/ACT and streams the rows out.

Host-side work is limited to index metadata (sharding, grouping edges by
destination tile, padding, bincount, scatter of comp values into one-hot
positions) and data layout (transpose/pad/quantize of input tensors).
All floating-point arithmetic on tensor data runs on device.
"""

import math
from contextlib import ExitStack
from dataclasses import dataclass

import numpy as np

import ml_dtypes

import concourse.bacc as bacc
import concourse.bass as bass
import concourse.mybir as mybir
import concourse.tile as tile
from concourse.bass_utils import run_bass_kernel_spmd

F32 = mybir.dt.float32
F8 = mybir.dt.float8e4
I16 = mybir.dt.int16
NP_F8 = ml_dtypes.float8_e4m3
P = 128  # SBUF partitions


@dataclass
class Cfg:
    n_nodes: int = 50000
    n_rel: int = 5
    n_bases: int = 5
    out_dim: int = 300
    n_cores: int = 8
    g_blk: int = 8          # gather-group size in 128-edge blocks (<=8: 1024 idx/call)
    tb_sz: int = 8          # tails per batched tail gather call
    scale: float = 32.0     # power-of-2 exponent shift for the fp8 basis table
    double_row: bool = True   # fp8 DoubleRow matmuls (2 edge blocks / matmul)
    swdge_queues: int = 4   # SWDGE queues to round-robin dma_gather over
    act_preload: bool = True  # preload the Exp+Ln act table set once

    @property
    def rowp(self) -> int:  # elements per fp8 basisQ row (256B multiple)
        return (self.row + 255) // 256 * 256

    @property
    def half(self) -> int:  # src-range half size for int16 gather indices
        return (self.n_nodes + 1) // 2

    @property
    def row(self) -> int:  # payload elements per basisQ row
        return self.n_bases * self.out_dim

    @property
    def lrow(self) -> int:  # elements per lhsT slab row (5 one-hot rows)
        return self.n_rel * P

    @property
    def npc(self) -> int:  # nodes per core, multiple of 128
        return ((self.n_nodes + self.n_cores - 1) // self.n_cores + P - 1) // P * P

    @property
    def n_tiles(self) -> int:
        return self.npc // P


def _c16(x):
    return (x + 15) // 16 * 16


class Plan:
    """Shared (cross-core) slot/slab layout derived from max counts."""

    def __init__(self, cfg: Cfg, counts):
        T = cfg.n_tiles
        self.cells = []  # dicts
        fgid = 0
        for t in range(T):
            for h in range(2):
                mx = int(counts[:, t, h].max())
                if mx == 0 and h == 1 and counts[:, t, 0].max() == 0:
                    mx = 16  # dummy tail keeps the tile's psum zeroed
                elif mx == 0:
                    continue
                fullB = (mx - 1) // P
                self.cells.append(dict(
                    t=t, h=h, fullB=fullB, mx=mx, fg0=fgid,
                    tlen=_c16(mx - fullB * P), toff=-1, colgid={},
                ))
                fgid += fullB
        self.NBF = fgid

        # tail batches per half, tiles in order
        tails = {0: [c for c in self.cells if c["h"] == 0],
                 1: [c for c in self.cells if c["h"] == 1]}
        self.batches = []
        soff = self.NBF * P
        gid = self.NBF
        kmax = max((len(v) + cfg.tb_sz - 1) // cfg.tb_sz for v in tails.values())
        for k in range(kmax):
            for h in (0, 1):
                grp = tails[h][k * cfg.tb_sz : (k + 1) * cfg.tb_sz]
                if not grp:
                    continue
                off = 0
                slabs = []
                g0 = gid
                for cell in grp:
                    cell["toff"] = soff + off
                    for jc in range(off // P, (off + cell["tlen"] - 1) // P + 1):
                        cell["colgid"][jc] = gid
                        slabs.append((cell["t"], jc, gid))
                        gid += 1
                    off += cell["tlen"]
                span = (off + P - 1) // P * P
                self.batches.append(dict(
                    h=h, soff=soff, nidx=span, g0=g0, slabs=slabs,
                    last_t=grp[-1]["t"], first_t=grp[0]["t"],
                ))
                soff += span
        self.NS = soff
        self.NSLAB = gid
        self.tile_slabs = [[] for _ in range(T)]
        for bi, bt in enumerate(self.batches):
            for (t, jc, g) in bt["slabs"]:
                self.tile_slabs[t].append((bi, jc, g))
        # prefetch: issue each batch one tb_sz-tile window BEFORE its first
        # consumer so its gather never sits on the critical chain
        self.issue_at = [[] for _ in range(T)]
        for bi, bt in enumerate(self.batches):
            self.issue_at[max(0, bt["first_t"] - cfg.tb_sz)].append(bi)
        self.max_slabs = max(len(bt["slabs"]) for bt in self.batches)


# ----------------------------------------------------------------------------
# Host-side prep: index metadata + layout only (no float math on tensor data)
# ----------------------------------------------------------------------------

def host_prep(cfg: Cfg, edge_index, edge_type, basis, root, bias, comp):
    src = np.asarray(edge_index[0]).astype(np.int64)
    dst = np.asarray(edge_index[1]).astype(np.int64)
    etype = np.asarray(edge_type).astype(np.int64)
    basis = np.asarray(basis, dtype=np.float32)
    root = np.asarray(root, dtype=np.float32)
    bias = np.asarray(bias, dtype=np.float32)
    comp = np.asarray(comp, dtype=np.float32)

    N, R, B, OUT = cfg.n_nodes, cfg.n_rel, cfg.n_bases, cfg.out_dim
    NPC, T = cfg.npc, cfg.n_tiles
    HALF = cfg.half

    basisQ = np.zeros((N, cfg.rowp), dtype=NP_F8)
    basisQ[:, : B * OUT] = (
        basis.transpose(1, 0, 2).reshape(N, B * OUT) * cfg.scale
    ).astype(NP_F8)
    comp_f8 = comp.astype(NP_F8)
    biast = np.ascontiguousarray(np.broadcast_to(bias[None, :], (P, OUT)))

    core_of = dst // NPC
    per_core = []
    counts = np.zeros((cfg.n_cores, T, 2), dtype=np.int64)
    for c in range(cfg.n_cores):
        m = core_of == c
        s_c, t_c, dl_c = src[m], etype[m], dst[m] - c * NPC
        tid = dl_c // P
        hid = s_c // HALF
        order = np.argsort(tid * 2 + hid, kind="stable")
        s_c, t_c, dl_c = s_c[order], t_c[order], dl_c[order]
        tid, hid = tid[order], hid[order]
        for t in range(T):
            for h in range(2):
                counts[c, t, h] = np.count_nonzero((tid == t) & (hid == h))
        per_core.append((s_c, t_c, dl_c))

    plan = Plan(cfg, counts)
    NS, NSLAB = plan.NS, plan.NSLAB

    in_maps = []
    for c in range(cfg.n_cores):
        s_c, t_c, dl_c = per_core[c]
        srcs = np.zeros((NS,), dtype=np.int64)
        typs = np.zeros((NS,), dtype=np.int64)
        dloc = np.zeros((NS,), dtype=np.int64)
        gids = np.zeros((NS,), dtype=np.int64)
        valid = np.zeros((NS,), dtype=bool)
        pos = np.concatenate([[0], np.cumsum(counts[c].reshape(-1))])
        for cell in plan.cells:
            t, h, fullB = cell["t"], cell["h"], cell["fullB"]
            gi = t * 2 + h
            a, b = pos[gi], pos[gi + 1]
            n = b - a
            if n == 0:
                continue
            q = min(n, fullB * P)
            fs = cell["fg0"] * P
            srcs[fs : fs + q] = s_c[a : a + q] - h * HALF
            typs[fs : fs + q] = t_c[a : a + q]
            dloc[fs : fs + q] = dl_c[a : a + q] - t * P
            gids[fs : fs + q] = cell["fg0"] + np.arange(q) // P
            valid[fs : fs + q] = True
            nt = n - q
            if nt > 0:
                ts0 = cell["toff"]
                abs_sl = ts0 + np.arange(nt)
                srcs[abs_sl] = s_c[a + q : b] - h * HALF
                typs[abs_sl] = t_c[a + q : b]
                dloc[abs_sl] = dl_c[a + q : b] - t * P
                bt = plan.batches[[i for i, x in enumerate(plan.batches)
                                   if x["soff"] <= ts0 < x["soff"] + x["nidx"]][0]]
                cols = (abs_sl - bt["soff"]) // P
                cg = cell["colgid"]
                gids[abs_sl] = np.array([cg[int(j)] for j in cols])
                valid[abs_sl] = True

        # comp-scaled one-hot lhsT slabs (pure scatter of comp values)
        lhsT = np.zeros((P, NSLAB, R, P), dtype=NP_F8)
        pv = np.arange(NS) % P
        lhsT[pv[valid], gids[valid], :, dloc[valid]] = comp_f8[typs[valid], :]

        idx16 = np.ascontiguousarray(
            np.tile(srcs.astype(np.int16).reshape(NS // 16, 16).T, (8, 1))
        )  # [128, NS/16] int16
        idx_cols = idx16.view(np.float32)  # [128, NS/32]

        cnt = np.zeros((NPC,), dtype=np.float32)
        np.add.at(cnt, dl_c, 1.0)
        cnt2 = cnt.reshape(T, P).T.copy()

        rootp = np.zeros((NPC, OUT), dtype=np.float32)
        lo, hi = c * NPC, min((c + 1) * NPC, N)
        if hi > lo:
            rootp[: hi - lo] = root[lo:hi]

        consts = np.concatenate([idx_cols, cnt2, biast], axis=1).copy()
        in_maps.append(dict(basisQ=basisQ, consts=consts, lhsT=lhsT, rootp=rootp))
    return plan, in_maps, bool(np.any(bias != 0.0))


# ----------------------------------------------------------------------------
# Device program
# ----------------------------------------------------------------------------

def build_program(cfg: Cfg, plan: Plan, use_bias: bool):
    N, R, OUT, ROW = cfg.n_nodes, cfg.n_rel, cfg.out_dim, cfg.rowp
    NPC, T = cfg.npc, cfg.n_tiles
    HALF = cfg.half
    LROW = cfg.lrow
    NS, NSLAB = plan.NS, plan.NSLAB
    IDXW = NS // 32  # f32 cols holding int16 gather indices

    nc = bacc.Bacc(
        "TRN2",
        target_bir_lowering=False,
        debug=False,
        enable_asserts=False,
        num_devices=cfg.n_cores,
        num_swdge_queues=cfg.swdge_queues,
    )
    W = IDXW + T + OUT
    basisQ = nc.dram_tensor("basisQ", [N, ROW], F8, kind="ExternalInput").ap()
    lhsT = nc.dram_tensor("lhsT", [P, NSLAB, LROW], F8, kind="ExternalInput").ap()
    consts = nc.dram_tensor("consts", [P, W], F32, kind="ExternalInput").ap()
    rootp = nc.dram_tensor("rootp", [NPC, OUT], F32, kind="ExternalInput").ap()
    out = nc.dram_tensor("out", [NPC, OUT], F32, kind="ExternalOutput").ap()

    add = mybir.AluOpType.add

    by_tile = [[] for _ in range(T)]
    for cell in plan.cells:
        by_tile[cell["t"]].append(cell)

    with tile.TileContext(nc) as tc, ExitStack() as ctx:
        cpool = ctx.enter_context(tc.tile_pool(name="const", bufs=1))
        gpool = ctx.enter_context(tc.tile_pool(name="g", bufs=5))
        lpool = ctx.enter_context(tc.tile_pool(name="lhs", bufs=4))
        tgpool = ctx.enter_context(tc.tile_pool(name="tg", bufs=4))
        tlpool = ctx.enter_context(tc.tile_pool(name="tl", bufs=4))
        ppool = ctx.enter_context(tc.tile_pool(name="ps", bufs=4, space="PSUM"))
        rpool = ctx.enter_context(tc.tile_pool(name="root", bufs=3))
        epool = ctx.enter_context(tc.tile_pool(name="epi", bufs=3))

        if cfg.act_preload:
            from concourse.hw_specs import get_activation_tables

            A = mybir.ActivationFunctionType
            tabs = list(get_activation_tables(nc.m.arch).values())
            set_id = next(i for i, s in enumerate(tabs) if A.Exp in s and A.Ln in s)
            pre = mybir.InstLoadActFuncSet(
                name=nc.get_next_instruction_name(),
                ins=[], outs=[], act_func_set_id=set_id,
            )
            pre.engine = mybir.EngineType.Activation
            nc.add_instruction(pre)

        consts_sb = cpool.tile([P, W], F32)
        c0 = min(8 * cfg.g_blk, IDXW)
        nc.sync.dma_start(consts_sb[:, 0:c0], consts[:, 0:c0])
        nc.sync.dma_start(consts_sb[:, c0:IDXW], consts[:, c0:IDXW])
        nc.sync.dma_start(consts_sb[:, IDXW:W], consts[:, IDXW:W])
        idx16_all = consts_sb[:, 0:IDXW].bitcast(I16)  # [128, NS/16]
        cnt_sb = consts_sb[:, IDXW : IDXW + T]
        biast_sb = consts_sb[:, IDXW + T : IDXW + T + OUT]
        rcnt_sb = cpool.tile([P, T], F32)
        nc.vector.tensor_scalar_max(rcnt_sb[:], cnt_sb, 1.0)
        nc.vector.reciprocal(rcnt_sb[:], rcnt_sb[:])
        nc.vector.tensor_scalar(
            out=rcnt_sb[:], in0=rcnt_sb[:], scalar1=1.0 / cfg.scale,
            scalar2=None, op0=mybir.AluOpType.mult,
        )

        def gather(soff, nidx, h, out_ap):
            nonlocal gq
            hi = min((h + 1) * HALF, N)
            nc.gpsimd.dma_gather(
                out_ap=out_ap,
                in_ap=basisQ[h * HALF : hi, :],
                idxs_ap=idx16_all[:, soff // 16 : (soff + nidx) // 16],
                num_idxs=nidx,
                num_idxs_reg=nidx,
                elem_size=ROW,
                queue_num=gq,
            )
            gq = (gq + 1) % cfg.swdge_queues

        gq = 0
        lq = 0
        live = {}  # batch idx -> (tgt, tlt, g0)
        for t in range(T):
            # prefetch tail batches one window ahead of their consumers
            for bi in plan.issue_at[t]:
                bt = plan.batches[bi]
                tgt = tgpool.tile([P, cfg.tb_sz, ROW], F8, tag="tg")
                nj = bt["nidx"] // P
                gather(bt["soff"], bt["nidx"], bt["h"], tgt[:, :nj, :])
                tlt = tlpool.tile([P, plan.max_slabs, R, P], F8, tag="tl")
                ns = len(bt["slabs"])
                leng = (nc.scalar, nc.sync)[lq % 2]
                lq += 1
                leng.dma_start(
                    tlt[:, :ns, :, :], lhsT[:, bt["g0"] : bt["g0"] + ns, :]
                )
                live[bi] = (tgt, tlt, bt["g0"])

            tile_cells = by_tile[t]
            slabs = plan.tile_slabs[t]
            psum_t = ppool.tile([P, OUT], F32)
            root_t = rpool.tile([P, OUT], F32)
            nc.sync.dma_start(root_t[:], rootp[t * P : (t + 1) * P, :])
            # pair adjacent slabs of the same tail (same batch, jc+1, g+1)
            # into DoubleRow matmuls
            spairs = []
            i = 0
            while i < len(slabs):
                if (cfg.double_row and i + 1 < len(slabs)
                        and slabs[i + 1][0] == slabs[i][0]
                        and slabs[i + 1][1] == slabs[i][1] + 1
                        and slabs[i + 1][2] == slabs[i][2] + 1):
                    spairs.append((slabs[i], True))
                    i += 2
                else:
                    spairs.append((slabs[i], False))
                    i += 1
            n_mm = 5 * len(spairs)
            for cell in tile_cells:
                fb = cell["fullB"]
                for g0 in range(0, fb, cfg.g_blk):
                    nb = min(cfg.g_blk, fb - g0)
                    n_mm += (nb // 2 + nb % 2) * R if cfg.double_row else nb * R
            done = 0
            for cell in tile_cells:
                fb, fg0, h = cell["fullB"], cell["fg0"], cell["h"]
                for g0 in range(0, fb, cfg.g_blk):
                    nb = min(cfg.g_blk, fb - g0)
                    gb = fg0 + g0
                    gt = gpool.tile([P, cfg.g_blk, ROW], F8, tag="g")
                    gather(gb * P, nb * P, h, gt[:, :nb, :])
                    lt = lpool.tile([P, cfg.g_blk, R, P], F8, tag="l")
                    leng = (nc.scalar, nc.sync)[lq % 2]
                    lq += 1
                    leng.dma_start(lt[:, :nb, :, :], lhsT[:, gb : gb + nb, :])
                    j = 0
                    while j < nb:
                        if cfg.double_row and j + 1 < nb:
                            for b in range(R):
                                nc.tensor.matmul(
                                    psum_t[:],
                                    lhsT=lt[:, j : j + 2, b, :],
                                    rhs=gt[:, j : j + 2, b * OUT : (b + 1) * OUT],
                                    start=(done == 0),
                                    stop=(done == n_mm - 1),
                                    perf_mode=mybir.MatmulPerfMode.DoubleRow,
                                )
                                done += 1
                            j += 2
                        else:
                            for b in range(R):
                                nc.tensor.matmul(
                                    psum_t[:],
                                    lhsT=lt[:, j, b, :],
                                    rhs=gt[:, j, b * OUT : (b + 1) * OUT],
                                    start=(done == 0),
                                    stop=(done == n_mm - 1),
                                )
                                done += 1
                            j += 1
            # tail slabs of this tile
            for ((bi, jc, g), dr) in spairs:
                tgt, tlt, g0 = live[bi]
                for b in range(R):
                    if dr:
                        nc.tensor.matmul(
                            psum_t[:],
                            lhsT=tlt[:, g - g0 : g - g0 + 2, b, :],
                            rhs=tgt[:, jc : jc + 2, b * OUT : (b + 1) * OUT],
                            start=(done == 0),
                            stop=(done == n_mm - 1),
                            perf_mode=mybir.MatmulPerfMode.DoubleRow,
                        )
                    else:
                        nc.tensor.matmul(
                            psum_t[:],
                            lhsT=tlt[:, g - g0, b, :],
                            rhs=tgt[:, jc, b * OUT : (b + 1) * OUT],
                            start=(done == 0),
                            stop=(done == n_mm - 1),
                        )
                    done += 1

            # epilogue: mean(+dequant) on ACT, +root(+bias), relu, log_softmax
            h_t = epool.tile([P, OUT], F32, tag="h")
            nc.scalar.activation(
                out=h_t[:], in_=psum_t[:],
                func=mybir.ActivationFunctionType.Copy,
                scale=rcnt_sb[:, t : t + 1],
            )
            nc.vector.tensor_tensor(out=h_t[:], in0=h_t[:], in1=root_t[:], op=add)
            if use_bias:
                nc.vector.tensor_tensor(
                    out=h_t[:], in0=h_t[:], in1=biast_sb[:], op=add
                )
            nc.vector.tensor_scalar_max(h_t[:], h_t[:], 0.0)
            mx = epool.tile([P, 2], F32, tag="mx")
            nc.vector.tensor_reduce(
                out=mx[:, 0:1], in_=h_t[:], axis=mybir.AxisListType.X,
                op=mybir.AluOpType.max, negate=True,
            )
            ex = epool.tile([P, OUT], F32, tag="ex")
            nc.scalar.activation(
                out=ex[:], in_=h_t[:], func=mybir.ActivationFunctionType.Exp,
                bias=mx[:, 0:1], scale=1.0, accum_out=mx[:, 1:2],
            )
            ln = epool.tile([P, 2], F32, tag="ln")
            nc.scalar.activation(
                out=ln[:, 0:1], in_=mx[:, 1:2], func=mybir.ActivationFunctionType.Ln,
            )
            tot = epool.tile([P, 2], F32, tag="tot")
            nc.vector.tensor_tensor(
                out=tot[:, 0:1], in0=mx[:, 0:1], in1=ln[:, 0:1],
                op=mybir.AluOpType.subtract,
            )
            o = epool.tile([P, OUT], F32, tag="o")
            nc.scalar.activation(
                out=o[:], in_=h_t[:],
                func=mybir.ActivationFunctionType.Identity,
                bias=tot[:, 0:1], scale=1.0,
            )
            nc.sync.dma_start(out[t * P : (t + 1) * P, :], o[:])
    nc.compile()
    return nc


# ----------------------------------------------------------------------------
# Entry point
# ----------------------------------------------------------------------------

def _run(cfg: Cfg, inputs: dict, trace: bool = False):
    plan, in_maps, use_bias = host_prep(
        cfg,
        inputs["edge_index"], inputs["edge_type"], inputs["basis"],
        inputs["root"], inputs["bias"], inputs["comp"],
    )
    nc = build_program(cfg, plan, use_bias)
    res = run_bass_kernel_spmd(
        nc, in_maps, core_ids=list(range(cfg.n_cores)), trace=trace,
    )
    parts = [res.results[c]["out"] for c in range(cfg.n_cores)]
    full = np.concatenate(parts, axis=0)[: cfg.n_nodes]
    return np.ascontiguousarray(full.astype(np.float32)), res


def kernel(**inputs) -> np.ndarray:
    cfg = Cfg()
    out, _ = _run(cfg, inputs)
    return out


# revision 39
# speedup vs baseline: 1.0113x; 1.0113x over previous
"""RGCN (basis-decomposition, one-hot features) message passing on 8 trn2 NeuronCores.

Math (per reference):
    weight[r] = sum_b comp[r,b] * basis[b]          # [R, N, OUT]
    msg_e     = weight[edge_type_e, src_e]          # [E, OUT]
    agg       = segment_sum(msg, dst) / max(cnt, 1) # mean aggregation
    out       = log_softmax(relu(agg + root + bias), axis=1)

Distribution strategy: shard edges by DESTINATION node range across the 8
cores (each core owns N/8 output rows), so no collectives are needed.

Per 128-edge block the device:
  1. gathers, per edge, the 5 basis rows of the src node from an fp8e4
     table (basisQ[n] = 32 * basis[:, n, :], 1536 B contiguous per node)
     via gpsimd dma_gather,
  2. accumulates psum[tile] += lhsT_b.T @ basisrows_b for b = 0..4, where
     lhsT_b[e, nd] = comp[type_e, b] * (dst_e == nd) is a host-prepared
     comp-scaled one-hot slab (pure scatter of comp values driven by the
     edge structure; no arithmetic), streamed sequentially from DRAM in
     fp8e4.  This keeps DVE free: no on-device one-hot construction.

The Q7 software descriptor generation of dma_gather has a large fixed
cost per call, so per-cell remainder blocks ("tails", <=128 rows) are
packed 16-aligned into per-half batches of 8 tails and fetched with ONE
gather call per batch, prefetched one 8-tile window ahead of their first
consumer; a tail that lands across a 128-slot column boundary simply
gets two tile-restricted lhsT slabs.  The epilogue (mean incl. the 1/32
dequant, +root, +bias, relu, log_softmax) runs fused per tile on
DVE/ACT and streams the rows out.

Host-side work is limited to index metadata (sharding, grouping edges by
destination tile, padding, bincount, scatter of comp values into one-hot
positions) and data layout (transpose/pad/quantize of input tensors).
All floating-point arithmetic on tensor data runs on device.
"""

import math
from contextlib import ExitStack
from dataclasses import dataclass

import numpy as np

import ml_dtypes

import concourse.bacc as bacc
import concourse.bass as bass
import concourse.mybir as mybir
import concourse.tile as tile
from concourse.bass_utils import run_bass_kernel_spmd

F32 = mybir.dt.float32
F8 = mybir.dt.float8e4
I16 = mybir.dt.int16
NP_F8 = ml_dtypes.float8_e4m3
P = 128  # SBUF partitions


@dataclass
class Cfg:
    n_nodes: int = 50000
    n_rel: int = 5
    n_bases: int = 5
    out_dim: int = 300
    n_cores: int = 8
    g_blk: int = 8          # gather-group size in 128-edge blocks (<=8: 1024 idx/call)
    tb_sz: int = 8          # tails per batched tail gather call
    scale: float = 32.0     # power-of-2 exponent shift for the fp8 basis table
    double_row: bool = True   # fp8 DoubleRow matmuls (2 edge blocks / matmul)
    swdge_queues: int = 4   # SWDGE queues to round-robin dma_gather over
    act_preload: bool = True  # preload the Exp+Ln act table set once

    @property
    def rowp(self) -> int:  # elements per fp8 basisQ row (256B multiple)
        return (self.row + 255) // 256 * 256

    @property
    def half(self) -> int:  # src-range half size for int16 gather indices
        return (self.n_nodes + 1) // 2

    @property
    def row(self) -> int:  # payload elements per basisQ row
        return self.n_bases * self.out_dim

    @property
    def lrow(self) -> int:  # elements per lhsT slab row (5 one-hot rows)
        return self.n_rel * P

    @property
    def npc(self) -> int:  # nodes per core, multiple of 128
        return ((self.n_nodes + self.n_cores - 1) // self.n_cores + P - 1) // P * P

    @property
    def n_tiles(self) -> int:
        return self.npc // P


def _c16(x):
    return (x + 15) // 16 * 16


class Plan:
    """Shared (cross-core) slot/slab layout derived from max counts."""

    def __init__(self, cfg: Cfg, counts):
        T = cfg.n_tiles
        self.cells = []  # dicts
        fgid = 0
        for t in range(T):
            for h in range(2):
                mx = int(counts[:, t, h].max())
                if mx == 0 and h == 1 and counts[:, t, 0].max() == 0:
                    mx = 16  # dummy tail keeps the tile's psum zeroed
                elif mx == 0:
                    continue
                fullB = (mx - 1) // P
                self.cells.append(dict(
                    t=t, h=h, fullB=fullB, mx=mx, fg0=fgid,
                    tlen=_c16(mx - fullB * P), toff=-1, colgid={},
                ))
                fgid += fullB
        self.NBF = fgid

        # tail batches per half, tiles in order
        tails = {0: [c for c in self.cells if c["h"] == 0],
                 1: [c for c in self.cells if c["h"] == 1]}
        self.batches = []
        soff = self.NBF * P
        gid = self.NBF
        kmax = max((len(v) + cfg.tb_sz - 1) // cfg.tb_sz for v in tails.values())
        for k in range(kmax):
            for h in (0, 1):
                grp = tails[h][k * cfg.tb_sz : (k + 1) * cfg.tb_sz]
                if not grp:
                    continue
                off = 0
                slabs = []
                g0 = gid
                for cell in grp:
                    cell["toff"] = soff + off
                    for jc in range(off // P, (off + cell["tlen"] - 1) // P + 1):
                        cell["colgid"][jc] = gid
                        slabs.append((cell["t"], jc, gid))
                        gid += 1
                    off += cell["tlen"]
                span = (off + P - 1) // P * P
                self.batches.append(dict(
                    h=h, soff=soff, nidx=span, g0=g0, slabs=slabs,
                    last_t=grp[-1]["t"], first_t=grp[0]["t"],
                ))
                soff += span
        self.NS = soff
        self.NSLAB = gid
        self.tile_slabs = [[] for _ in range(T)]
        for bi, bt in enumerate(self.batches):
            for (t, jc, g) in bt["slabs"]:
                self.tile_slabs[t].append((bi, jc, g))
        # prefetch: issue each batch one tb_sz-tile window BEFORE its first
        # consumer so its gather never sits on the critical chain
        self.issue_at = [[] for _ in range(T)]
        for bi, bt in enumerate(self.batches):
            self.issue_at[max(0, bt["first_t"] - cfg.tb_sz)].append(bi)
        self.max_slabs = max(len(bt["slabs"]) for bt in self.batches)


# ----------------------------------------------------------------------------
# Host-side prep: index metadata + layout only (no float math on tensor data)
# ----------------------------------------------------------------------------

def host_prep(cfg: Cfg, edge_index, edge_type, basis, root, bias, comp):
    src = np.asarray(edge_index[0]).astype(np.int64)
    dst = np.asarray(edge_index[1]).astype(np.int64)
    etype = np.asarray(edge_type).astype(np.int64)
    basis = np.asarray(basis, dtype=np.float32)
    root = np.asarray(root, dtype=np.float32)
    bias = np.asarray(bias, dtype=np.float32)
    comp = np.asarray(comp, dtype=np.float32)

    N, R, B, OUT = cfg.n_nodes, cfg.n_rel, cfg.n_bases, cfg.out_dim
    NPC, T = cfg.npc, cfg.n_tiles
    HALF = cfg.half

    basisQ = np.zeros((N, cfg.rowp), dtype=NP_F8)
    basisQ[:, : B * OUT] = (
        basis.transpose(1, 0, 2).reshape(N, B * OUT) * cfg.scale
    ).astype(NP_F8)
    comp_f8 = comp.astype(NP_F8)
    biast = np.ascontiguousarray(np.broadcast_to(bias[None, :], (P, OUT)))

    core_of = dst // NPC
    per_core = []
    counts = np.zeros((cfg.n_cores, T, 2), dtype=np.int64)
    for c in range(cfg.n_cores):
        m = core_of == c
        s_c, t_c, dl_c = src[m], etype[m], dst[m] - c * NPC
        tid = dl_c // P
        hid = s_c // HALF
        order = np.argsort(tid * 2 + hid, kind="stable")
        s_c, t_c, dl_c = s_c[order], t_c[order], dl_c[order]
        tid, hid = tid[order], hid[order]
        for t in range(T):
            for h in range(2):
                counts[c, t, h] = np.count_nonzero((tid == t) & (hid == h))
        per_core.append((s_c, t_c, dl_c))

    plan = Plan(cfg, counts)
    NS, NSLAB = plan.NS, plan.NSLAB

    in_maps = []
    for c in range(cfg.n_cores):
        s_c, t_c, dl_c = per_core[c]
        srcs = np.zeros((NS,), dtype=np.int64)
        typs = np.zeros((NS,), dtype=np.int64)
        dloc = np.zeros((NS,), dtype=np.int64)
        gids = np.zeros((NS,), dtype=np.int64)
        valid = np.zeros((NS,), dtype=bool)
        pos = np.concatenate([[0], np.cumsum(counts[c].reshape(-1))])
        for cell in plan.cells:
            t, h, fullB = cell["t"], cell["h"], cell["fullB"]
            gi = t * 2 + h
            a, b = pos[gi], pos[gi + 1]
            n = b - a
            if n == 0:
                continue
            q = min(n, fullB * P)
            fs = cell["fg0"] * P
            srcs[fs : fs + q] = s_c[a : a + q] - h * HALF
            typs[fs : fs + q] = t_c[a : a + q]
            dloc[fs : fs + q] = dl_c[a : a + q] - t * P
            gids[fs : fs + q] = cell["fg0"] + np.arange(q) // P
            valid[fs : fs + q] = True
            nt = n - q
            if nt > 0:
                ts0 = cell["toff"]
                abs_sl = ts0 + np.arange(nt)
                srcs[abs_sl] = s_c[a + q : b] - h * HALF
                typs[abs_sl] = t_c[a + q : b]
                dloc[abs_sl] = dl_c[a + q : b] - t * P
                bt = plan.batches[[i for i, x in enumerate(plan.batches)
                                   if x["soff"] <= ts0 < x["soff"] + x["nidx"]][0]]
                cols = (abs_sl - bt["soff"]) // P
                cg = cell["colgid"]
                gids[abs_sl] = np.array([cg[int(j)] for j in cols])
                valid[abs_sl] = True

        # comp-scaled one-hot lhsT slabs (pure scatter of comp values)
        lhsT = np.zeros((P, NSLAB, R, P), dtype=NP_F8)
        pv = np.arange(NS) % P
        lhsT[pv[valid], gids[valid], :, dloc[valid]] = comp_f8[typs[valid], :]

        idx16 = np.ascontiguousarray(
            np.tile(srcs.astype(np.int16).reshape(NS // 16, 16).T, (8, 1))
        )  # [128, NS/16] int16
        idx_cols = idx16.view(np.float32)  # [128, NS/32]

        cnt = np.zeros((NPC,), dtype=np.float32)
        np.add.at(cnt, dl_c, 1.0)
        cnt2 = cnt.reshape(T, P).T.copy()

        rootp = np.zeros((NPC, OUT), dtype=np.float32)
        lo, hi = c * NPC, min((c + 1) * NPC, N)
        if hi > lo:
            rootp[: hi - lo] = root[lo:hi]

        consts = np.concatenate([idx_cols, cnt2, biast], axis=1).copy()
        in_maps.append(dict(basisQ=basisQ, consts=consts, lhsT=lhsT, rootp=rootp))
    return plan, in_maps, bool(np.any(bias != 0.0))


# ----------------------------------------------------------------------------
# Device program
# ----------------------------------------------------------------------------

def build_program(cfg: Cfg, plan: Plan, use_bias: bool):
    N, R, OUT, ROW = cfg.n_nodes, cfg.n_rel, cfg.out_dim, cfg.rowp
    NPC, T = cfg.npc, cfg.n_tiles
    HALF = cfg.half
    LROW = cfg.lrow
    NS, NSLAB = plan.NS, plan.NSLAB
    IDXW = NS // 32  # f32 cols holding int16 gather indices

    nc = bacc.Bacc(
        "TRN2",
        target_bir_lowering=False,
        debug=False,
        enable_asserts=False,
        num_devices=cfg.n_cores,
        num_swdge_queues=cfg.swdge_queues,
    )
    W = IDXW + T + OUT
    basisQ = nc.dram_tensor("basisQ", [N, ROW], F8, kind="ExternalInput").ap()
    lhsT = nc.dram_tensor("lhsT", [P, NSLAB, LROW], F8, kind="ExternalInput").ap()
    consts = nc.dram_tensor("consts", [P, W], F32, kind="ExternalInput").ap()
    rootp = nc.dram_tensor("rootp", [NPC, OUT], F32, kind="ExternalInput").ap()
    out = nc.dram_tensor("out", [NPC, OUT], F32, kind="ExternalOutput").ap()

    add = mybir.AluOpType.add

    by_tile = [[] for _ in range(T)]
    for cell in plan.cells:
        by_tile[cell["t"]].append(cell)

    with tile.TileContext(nc) as tc, ExitStack() as ctx:
        cpool = ctx.enter_context(tc.tile_pool(name="const", bufs=1))
        gpool = ctx.enter_context(tc.tile_pool(name="g", bufs=6))
        lpool = ctx.enter_context(tc.tile_pool(name="lhs", bufs=5))
        tgpool = ctx.enter_context(tc.tile_pool(name="tg", bufs=4))
        tlpool = ctx.enter_context(tc.tile_pool(name="tl", bufs=4))
        ppool = ctx.enter_context(tc.tile_pool(name="ps", bufs=4, space="PSUM"))
        rpool = ctx.enter_context(tc.tile_pool(name="root", bufs=3))
        epool = ctx.enter_context(tc.tile_pool(name="epi", bufs=3))

        if cfg.act_preload:
            from concourse.hw_specs import get_activation_tables

            A = mybir.ActivationFunctionType
            tabs = list(get_activation_tables(nc.m.arch).values())
            set_id = next(i for i, s in enumerate(tabs) if A.Exp in s and A.Ln in s)
            pre = mybir.InstLoadActFuncSet(
                name=nc.get_next_instruction_name(),
                ins=[], outs=[], act_func_set_id=set_id,
            )
            pre.engine = mybir.EngineType.Activation
            nc.add_instruction(pre)

        consts_sb = cpool.tile([P, W], F32)
        c0 = min(8 * cfg.g_blk, IDXW)
        nc.sync.dma_start(consts_sb[:, 0:c0], consts[:, 0:c0])
        nc.sync.dma_start(consts_sb[:, c0:IDXW], consts[:, c0:IDXW])
        nc.sync.dma_start(consts_sb[:, IDXW:W], consts[:, IDXW:W])
        idx16_all = consts_sb[:, 0:IDXW].bitcast(I16)  # [128, NS/16]
        cnt_sb = consts_sb[:, IDXW : IDXW + T]
        biast_sb = consts_sb[:, IDXW + T : IDXW + T + OUT]
        rcnt_sb = cpool.tile([P, T], F32)
        nc.vector.tensor_scalar_max(rcnt_sb[:], cnt_sb, 1.0)
        nc.vector.reciprocal(rcnt_sb[:], rcnt_sb[:])
        nc.vector.tensor_scalar(
            out=rcnt_sb[:], in0=rcnt_sb[:], scalar1=1.0 / cfg.scale,
            scalar2=None, op0=mybir.AluOpType.mult,
        )

        def gather(soff, nidx, h, out_ap):
            nonlocal gq
            hi = min((h + 1) * HALF, N)
            nc.gpsimd.dma_gather(
                out_ap=out_ap,
                in_ap=basisQ[h * HALF : hi, :],
                idxs_ap=idx16_all[:, soff // 16 : (soff + nidx) // 16],
                num_idxs=nidx,
                num_idxs_reg=nidx,
                elem_size=ROW,
                queue_num=gq,
            )
            gq = (gq + 1) % cfg.swdge_queues

        gq = 0
        lq = 0
        live = {}  # batch idx -> (tgt, tlt, g0)
        for t in range(T):
            # prefetch tail batches one window ahead of their consumers
            for bi in plan.issue_at[t]:
                bt = plan.batches[bi]
                tgt = tgpool.tile([P, cfg.tb_sz, ROW], F8, tag="tg")
                nj = bt["nidx"] // P
                gather(bt["soff"], bt["nidx"], bt["h"], tgt[:, :nj, :])
                tlt = tlpool.tile([P, plan.max_slabs, R, P], F8, tag="tl")
                ns = len(bt["slabs"])
                leng = (nc.scalar, nc.sync)[lq % 2]
                lq += 1
                leng.dma_start(
                    tlt[:, :ns, :, :], lhsT[:, bt["g0"] : bt["g0"] + ns, :]
                )
                live[bi] = (tgt, tlt, bt["g0"])

            tile_cells = by_tile[t]
            slabs = plan.tile_slabs[t]
            psum_t = ppool.tile([P, OUT], F32)
            root_t = rpool.tile([P, OUT], F32)
            nc.sync.dma_start(root_t[:], rootp[t * P : (t + 1) * P, :])
            n_mm = 5 * len(slabs)
            for cell in tile_cells:
                fb = cell["fullB"]
                for g0 in range(0, fb, cfg.g_blk):
                    nb = min(cfg.g_blk, fb - g0)
                    n_mm += (nb // 2 + nb % 2) * R if cfg.double_row else nb * R
            done = 0
            for cell in tile_cells:
                fb, fg0, h = cell["fullB"], cell["fg0"], cell["h"]
                for g0 in range(0, fb, cfg.g_blk):
                    nb = min(cfg.g_blk, fb - g0)
                    gb = fg0 + g0
                    gt = gpool.tile([P, cfg.g_blk, ROW], F8, tag="g")
                    gather(gb * P, nb * P, h, gt[:, :nb, :])
                    lt = lpool.tile([P, cfg.g_blk, R, P], F8, tag="l")
                    leng = (nc.scalar, nc.sync)[lq % 2]
                    lq += 1
                    leng.dma_start(lt[:, :nb, :, :], lhsT[:, gb : gb + nb, :])
                    j = 0
                    while j < nb:
                        if cfg.double_row and j + 1 < nb:
                            for b in range(R):
                                nc.tensor.matmul(
                                    psum_t[:],
                                    lhsT=lt[:, j : j + 2, b, :],
                                    rhs=gt[:, j : j + 2, b * OUT : (b + 1) * OUT],
                                    start=(done == 0),
                                    stop=(done == n_mm - 1),
                                    perf_mode=mybir.MatmulPerfMode.DoubleRow,
                                )
                                done += 1
                            j += 2
                        else:
                            for b in range(R):
                                nc.tensor.matmul(
                                    psum_t[:],
                                    lhsT=lt[:, j, b, :],
                                    rhs=gt[:, j, b * OUT : (b + 1) * OUT],
                                    start=(done == 0),
                                    stop=(done == n_mm - 1),
                                )
                                done += 1
                            j += 1
            # tail slabs of this tile
            for (bi, jc, g) in slabs:
                tgt, tlt, g0 = live[bi]
                for b in range(R):
                    nc.tensor.matmul(
                        psum_t[:],
                        lhsT=tlt[:, g - g0, b, :],
                        rhs=tgt[:, jc, b * OUT : (b + 1) * OUT],
                        start=(done == 0),
                        stop=(done == n_mm - 1),
                    )
                    done += 1

            # epilogue: mean(+dequant) on ACT, +root(+bias), relu, log_softmax
            h_t = epool.tile([P, OUT], F32, tag="h")
            nc.scalar.activation(
                out=h_t[:], in_=psum_t[:],
                func=mybir.ActivationFunctionType.Copy,
                scale=rcnt_sb[:, t : t + 1],
            )
            nc.vector.tensor_tensor(out=h_t[:], in0=h_t[:], in1=root_t[:], op=add)
            if use_bias:
                nc.vector.tensor_tensor(
                    out=h_t[:], in0=h_t[:], in1=biast_sb[:], op=add
                )
            nc.vector.tensor_scalar_max(h_t[:], h_t[:], 0.0)
            mx = epool.tile([P, 2], F32, tag="mx")
            nc.vector.tensor_reduce(
                out=mx[:, 0:1], in_=h_t[:], axis=mybir.AxisListType.X,
                op=mybir.AluOpType.max, negate=True,
            )
            ex = epool.tile([P, OUT], F32, tag="ex")
            nc.scalar.activation(
                out=ex[:], in_=h_t[:], func=mybir.ActivationFunctionType.Exp,
                bias=mx[:, 0:1], scale=1.0, accum_out=mx[:, 1:2],
            )
            ln = epool.tile([P, 2], F32, tag="ln")
            nc.scalar.activation(
                out=ln[:, 0:1], in_=mx[:, 1:2], func=mybir.ActivationFunctionType.Ln,
            )
            tot = epool.tile([P, 2], F32, tag="tot")
            nc.vector.tensor_tensor(
                out=tot[:, 0:1], in0=mx[:, 0:1], in1=ln[:, 0:1],
                op=mybir.AluOpType.subtract,
            )
            o = epool.tile([P, OUT], F32, tag="o")
            nc.scalar.activation(
                out=o[:], in_=h_t[:],
                func=mybir.ActivationFunctionType.Identity,
                bias=tot[:, 0:1], scale=1.0,
            )
            nc.sync.dma_start(out[t * P : (t + 1) * P, :], o[:])
    nc.compile()
    return nc


# ----------------------------------------------------------------------------
# Entry point
# ----------------------------------------------------------------------------

def _run(cfg: Cfg, inputs: dict, trace: bool = False):
    plan, in_maps, use_bias = host_prep(
        cfg,
        inputs["edge_index"], inputs["edge_type"], inputs["basis"],
        inputs["root"], inputs["bias"], inputs["comp"],
    )
    nc = build_program(cfg, plan, use_bias)
    res = run_bass_kernel_spmd(
        nc, in_maps, core_ids=list(range(cfg.n_cores)), trace=trace,
    )
    parts = [res.results[c]["out"] for c in range(cfg.n_cores)]
    full = np.concatenate(parts, axis=0)[: cfg.n_nodes]
    return np.ascontiguousarray(full.astype(np.float32)), res


def kernel(**inputs) -> np.ndarray:
    cfg = Cfg()
    out, _ = _run(cfg, inputs)
    return out


# revision 41
# speedup vs baseline: 1.0598x; 1.0480x over previous
"""RGCN (basis-decomposition, one-hot features) message passing on 8 trn2 NeuronCores.

Math (per reference):
    weight[r] = sum_b comp[r,b] * basis[b]          # [R, N, OUT]
    msg_e     = weight[edge_type_e, src_e]          # [E, OUT]
    agg       = segment_sum(msg, dst) / max(cnt, 1) # mean aggregation
    out       = log_softmax(relu(agg + root + bias), axis=1)

Distribution strategy: shard edges by DESTINATION node range across the 8
cores (each core owns N/8 output rows), so no collectives are needed.

Per 128-edge block the device:
  1. gathers, per edge, the 5 basis rows of the src node from an fp8e4
     table (basisQ[n] = 32 * basis[:, n, :], 1536 B contiguous per node)
     via gpsimd dma_gather,
  2. accumulates psum[tile] += lhsT_b.T @ basisrows_b for b = 0..4, where
     lhsT_b[e, nd] = comp[type_e, b] * (dst_e == nd) is a host-prepared
     comp-scaled one-hot slab (pure scatter of comp values driven by the
     edge structure; no arithmetic), streamed sequentially from DRAM in
     fp8e4.  This keeps DVE free: no on-device one-hot construction.

The Q7 software descriptor generation of dma_gather has a large fixed
cost per call, so per-cell remainder blocks ("tails", <=128 rows) are
packed 16-aligned into per-half batches of 8 tails and fetched with ONE
gather call per batch, prefetched one 8-tile window ahead of their first
consumer; a tail that lands across a 128-slot column boundary simply
gets two tile-restricted lhsT slabs.  The epilogue (mean incl. the 1/32
dequant, +root, +bias, relu, log_softmax) runs fused per tile on
DVE/ACT and streams the rows out.

Host-side work is limited to index metadata (sharding, grouping edges by
destination tile, padding, bincount, scatter of comp values into one-hot
positions) and data layout (transpose/pad/quantize of input tensors).
All floating-point arithmetic on tensor data runs on device.
"""

import math
from contextlib import ExitStack
from dataclasses import dataclass

import numpy as np

import ml_dtypes

import concourse.bacc as bacc
import concourse.bass as bass
import concourse.mybir as mybir
import concourse.tile as tile
from concourse.bass_utils import run_bass_kernel_spmd

F32 = mybir.dt.float32
F8 = mybir.dt.float8e4
I16 = mybir.dt.int16
NP_F8 = ml_dtypes.float8_e4m3
P = 128  # SBUF partitions


@dataclass
class Cfg:
    n_nodes: int = 50000
    n_rel: int = 5
    n_bases: int = 5
    out_dim: int = 300
    n_cores: int = 8
    g_blk: int = 8          # gather-group size in 128-edge blocks (<=8: 1024 idx/call)
    tb_sz: int = 8          # tails per batched tail gather call
    scale: float = 32.0     # power-of-2 exponent shift for the fp8 basis table
    double_row: bool = True   # fp8 DoubleRow matmuls (2 edge blocks / matmul)
    swdge_queues: int = 4   # SWDGE queues to round-robin dma_gather over
    act_preload: bool = True  # preload the Exp+Ln act table set once

    @property
    def rowp(self) -> int:  # elements per fp8 basisQ row (256B multiple)
        return (self.row + 255) // 256 * 256

    @property
    def half(self) -> int:  # src-range half size for int16 gather indices
        return (self.n_nodes + 1) // 2

    @property
    def row(self) -> int:  # payload elements per basisQ row
        return self.n_bases * self.out_dim

    @property
    def lrow(self) -> int:  # elements per lhsT slab row (5 one-hot rows)
        return self.n_rel * P

    @property
    def npc(self) -> int:  # nodes per core, multiple of 128
        return ((self.n_nodes + self.n_cores - 1) // self.n_cores + P - 1) // P * P

    @property
    def n_tiles(self) -> int:
        return self.npc // P


def _c16(x):
    return (x + 15) // 16 * 16


class Plan:
    """Shared (cross-core) slot/slab layout derived from max counts."""

    def __init__(self, cfg: Cfg, counts):
        T = cfg.n_tiles
        self.cells = []  # dicts
        fgid = 0
        for t in range(T):
            for h in range(2):
                mx = int(counts[:, t, h].max())
                if mx == 0 and h == 1 and counts[:, t, 0].max() == 0:
                    mx = 16  # dummy tail keeps the tile's psum zeroed
                elif mx == 0:
                    continue
                fullB = (mx - 1) // P
                self.cells.append(dict(
                    t=t, h=h, fullB=fullB, mx=mx, fg0=fgid,
                    tlen=_c16(mx - fullB * P), toff=-1, colgid={},
                ))
                fgid += fullB
        self.NBF = fgid

        # tail batches per half, tiles in order
        tails = {0: [c for c in self.cells if c["h"] == 0],
                 1: [c for c in self.cells if c["h"] == 1]}
        self.batches = []
        soff = self.NBF * P
        gid = self.NBF
        kmax = max((len(v) + cfg.tb_sz - 1) // cfg.tb_sz for v in tails.values())
        for k in range(kmax):
            for h in (0, 1):
                grp = tails[h][k * cfg.tb_sz : (k + 1) * cfg.tb_sz]
                if not grp:
                    continue
                off = 0
                slabs = []
                g0 = gid
                for cell in grp:
                    cell["toff"] = soff + off
                    for jc in range(off // P, (off + cell["tlen"] - 1) // P + 1):
                        cell["colgid"][jc] = gid
                        slabs.append((cell["t"], jc, gid))
                        gid += 1
                    off += cell["tlen"]
                span = (off + P - 1) // P * P
                self.batches.append(dict(
                    h=h, soff=soff, nidx=span, g0=g0, slabs=slabs,
                    last_t=grp[-1]["t"], first_t=grp[0]["t"],
                ))
                soff += span
        self.NS = soff
        self.NSLAB = gid
        self.tile_slabs = [[] for _ in range(T)]
        for bi, bt in enumerate(self.batches):
            for (t, jc, g) in bt["slabs"]:
                self.tile_slabs[t].append((bi, jc, g))
        # prefetch: issue each batch one tb_sz-tile window BEFORE its first
        # consumer so its gather never sits on the critical chain
        self.issue_at = [[] for _ in range(T)]
        for bi, bt in enumerate(self.batches):
            self.issue_at[max(0, bt["first_t"] - cfg.tb_sz)].append(bi)
        self.max_slabs = max(len(bt["slabs"]) for bt in self.batches)


# ----------------------------------------------------------------------------
# Host-side prep: index metadata + layout only (no float math on tensor data)
# ----------------------------------------------------------------------------

def host_prep(cfg: Cfg, edge_index, edge_type, basis, root, bias, comp):
    src = np.asarray(edge_index[0]).astype(np.int64)
    dst = np.asarray(edge_index[1]).astype(np.int64)
    etype = np.asarray(edge_type).astype(np.int64)
    basis = np.asarray(basis, dtype=np.float32)
    root = np.asarray(root, dtype=np.float32)
    bias = np.asarray(bias, dtype=np.float32)
    comp = np.asarray(comp, dtype=np.float32)

    N, R, B, OUT = cfg.n_nodes, cfg.n_rel, cfg.n_bases, cfg.out_dim
    NPC, T = cfg.npc, cfg.n_tiles
    HALF = cfg.half

    basisQ = np.zeros((N, cfg.rowp), dtype=NP_F8)
    basisQ[:, : B * OUT] = (
        basis.transpose(1, 0, 2).reshape(N, B * OUT) * cfg.scale
    ).astype(NP_F8)
    comp_f8 = comp.astype(NP_F8)
    biast = np.ascontiguousarray(np.broadcast_to(bias[None, :], (P, OUT)))

    core_of = dst // NPC
    per_core = []
    counts = np.zeros((cfg.n_cores, T, 2), dtype=np.int64)
    for c in range(cfg.n_cores):
        m = core_of == c
        s_c, t_c, dl_c = src[m], etype[m], dst[m] - c * NPC
        tid = dl_c // P
        hid = s_c // HALF
        order = np.argsort(tid * 2 + hid, kind="stable")
        s_c, t_c, dl_c = s_c[order], t_c[order], dl_c[order]
        tid, hid = tid[order], hid[order]
        for t in range(T):
            for h in range(2):
                counts[c, t, h] = np.count_nonzero((tid == t) & (hid == h))
        per_core.append((s_c, t_c, dl_c))

    plan = Plan(cfg, counts)
    NS, NSLAB = plan.NS, plan.NSLAB

    in_maps = []
    for c in range(cfg.n_cores):
        s_c, t_c, dl_c = per_core[c]
        srcs = np.zeros((NS,), dtype=np.int64)
        typs = np.zeros((NS,), dtype=np.int64)
        dloc = np.zeros((NS,), dtype=np.int64)
        gids = np.zeros((NS,), dtype=np.int64)
        valid = np.zeros((NS,), dtype=bool)
        pos = np.concatenate([[0], np.cumsum(counts[c].reshape(-1))])
        for cell in plan.cells:
            t, h, fullB = cell["t"], cell["h"], cell["fullB"]
            gi = t * 2 + h
            a, b = pos[gi], pos[gi + 1]
            n = b - a
            if n == 0:
                continue
            q = min(n, fullB * P)
            fs = cell["fg0"] * P
            srcs[fs : fs + q] = s_c[a : a + q] - h * HALF
            typs[fs : fs + q] = t_c[a : a + q]
            dloc[fs : fs + q] = dl_c[a : a + q] - t * P
            gids[fs : fs + q] = cell["fg0"] + np.arange(q) // P
            valid[fs : fs + q] = True
            nt = n - q
            if nt > 0:
                ts0 = cell["toff"]
                abs_sl = ts0 + np.arange(nt)
                srcs[abs_sl] = s_c[a + q : b] - h * HALF
                typs[abs_sl] = t_c[a + q : b]
                dloc[abs_sl] = dl_c[a + q : b] - t * P
                bt = plan.batches[[i for i, x in enumerate(plan.batches)
                                   if x["soff"] <= ts0 < x["soff"] + x["nidx"]][0]]
                cols = (abs_sl - bt["soff"]) // P
                cg = cell["colgid"]
                gids[abs_sl] = np.array([cg[int(j)] for j in cols])
                valid[abs_sl] = True

        # comp-scaled one-hot lhsT slabs (pure scatter of comp values)
        lhsT = np.zeros((P, NSLAB, R, P), dtype=NP_F8)
        pv = np.arange(NS) % P
        lhsT[pv[valid], gids[valid], :, dloc[valid]] = comp_f8[typs[valid], :]

        idx16 = np.ascontiguousarray(
            np.tile(srcs.astype(np.int16).reshape(NS // 16, 16).T, (8, 1))
        )  # [128, NS/16] int16
        idx_cols = idx16.view(np.float32)  # [128, NS/32]

        cnt = np.zeros((NPC,), dtype=np.float32)
        np.add.at(cnt, dl_c, 1.0)
        cnt2 = cnt.reshape(T, P).T.copy()

        rootp = np.zeros((NPC, OUT), dtype=np.float32)
        lo, hi = c * NPC, min((c + 1) * NPC, N)
        if hi > lo:
            rootp[: hi - lo] = root[lo:hi]

        consts = np.concatenate([idx_cols, cnt2, biast], axis=1).copy()
        in_maps.append(dict(basisQ=basisQ, consts=consts, lhsT=lhsT, rootp=rootp))
    return plan, in_maps, bool(np.any(bias != 0.0))


# ----------------------------------------------------------------------------
# Device program
# ----------------------------------------------------------------------------

def build_program(cfg: Cfg, plan: Plan, use_bias: bool):
    N, R, OUT, ROW = cfg.n_nodes, cfg.n_rel, cfg.out_dim, cfg.rowp
    NPC, T = cfg.npc, cfg.n_tiles
    HALF = cfg.half
    LROW = cfg.lrow
    NS, NSLAB = plan.NS, plan.NSLAB
    IDXW = NS // 32  # f32 cols holding int16 gather indices

    nc = bacc.Bacc(
        "TRN2",
        target_bir_lowering=False,
        debug=False,
        enable_asserts=False,
        num_devices=cfg.n_cores,
        num_swdge_queues=cfg.swdge_queues,
    )
    W = IDXW + T + OUT
    basisQ = nc.dram_tensor("basisQ", [N, ROW], F8, kind="ExternalInput").ap()
    lhsT = nc.dram_tensor("lhsT", [P, NSLAB, LROW], F8, kind="ExternalInput").ap()
    consts = nc.dram_tensor("consts", [P, W], F32, kind="ExternalInput").ap()
    rootp = nc.dram_tensor("rootp", [NPC, OUT], F32, kind="ExternalInput").ap()
    out = nc.dram_tensor("out", [NPC, OUT], F32, kind="ExternalOutput").ap()

    add = mybir.AluOpType.add

    by_tile = [[] for _ in range(T)]
    for cell in plan.cells:
        by_tile[cell["t"]].append(cell)

    with tile.TileContext(nc) as tc, ExitStack() as ctx:
        cpool = ctx.enter_context(tc.tile_pool(name="const", bufs=1))
        gpool = ctx.enter_context(tc.tile_pool(name="g", bufs=5))
        lpool = ctx.enter_context(tc.tile_pool(name="lhs", bufs=4))
        tgpool = ctx.enter_context(tc.tile_pool(name="tg", bufs=4))
        tlpool = ctx.enter_context(tc.tile_pool(name="tl", bufs=4))
        ppool = ctx.enter_context(tc.tile_pool(name="ps", bufs=4, space="PSUM"))
        rpool = ctx.enter_context(tc.tile_pool(name="root", bufs=3))
        epool = ctx.enter_context(tc.tile_pool(name="epi", bufs=3))

        if cfg.act_preload:
            from concourse.hw_specs import get_activation_tables

            A = mybir.ActivationFunctionType
            tabs = list(get_activation_tables(nc.m.arch).values())
            set_id = next(i for i, s in enumerate(tabs) if A.Exp in s and A.Ln in s)
            pre = mybir.InstLoadActFuncSet(
                name=nc.get_next_instruction_name(),
                ins=[], outs=[], act_func_set_id=set_id,
            )
            pre.engine = mybir.EngineType.Activation
            nc.add_instruction(pre)

        consts_sb = cpool.tile([P, W], F32)
        c0 = min(8 * cfg.g_blk, IDXW)
        nc.scalar.dma_start(consts_sb[:, 0:c0], consts[:, 0:c0])
        nc.scalar.dma_start(consts_sb[:, c0:IDXW], consts[:, c0:IDXW])
        nc.scalar.dma_start(consts_sb[:, IDXW:W], consts[:, IDXW:W])
        idx16_all = consts_sb[:, 0:IDXW].bitcast(I16)  # [128, NS/16]
        cnt_sb = consts_sb[:, IDXW : IDXW + T]
        biast_sb = consts_sb[:, IDXW + T : IDXW + T + OUT]
        rcnt_sb = cpool.tile([P, T], F32)
        nc.vector.tensor_scalar_max(rcnt_sb[:], cnt_sb, 1.0)
        nc.vector.reciprocal(rcnt_sb[:], rcnt_sb[:])
        nc.vector.tensor_scalar(
            out=rcnt_sb[:], in0=rcnt_sb[:], scalar1=1.0 / cfg.scale,
            scalar2=None, op0=mybir.AluOpType.mult,
        )

        def gather(soff, nidx, h, out_ap):
            nonlocal gq
            hi = min((h + 1) * HALF, N)
            nc.gpsimd.dma_gather(
                out_ap=out_ap,
                in_ap=basisQ[h * HALF : hi, :],
                idxs_ap=idx16_all[:, soff // 16 : (soff + nidx) // 16],
                num_idxs=nidx,
                num_idxs_reg=nidx,
                elem_size=ROW,
                queue_num=gq,
            )
            gq = (gq + 1) % cfg.swdge_queues

        gq = 0
        lq = 0
        live = {}  # batch idx -> (tgt, tlt, g0)
        for t in range(T):
            # prefetch tail batches one window ahead of their consumers
            for bi in plan.issue_at[t]:
                bt = plan.batches[bi]
                tgt = tgpool.tile([P, cfg.tb_sz, ROW], F8, tag="tg")
                nj = bt["nidx"] // P
                gather(bt["soff"], bt["nidx"], bt["h"], tgt[:, :nj, :])
                tlt = tlpool.tile([P, plan.max_slabs, R, P], F8, tag="tl")
                ns = len(bt["slabs"])
                nc.scalar.dma_start(
                    tlt[:, :ns, :, :], lhsT[:, bt["g0"] : bt["g0"] + ns, :]
                )
                live[bi] = (tgt, tlt, bt["g0"])

            tile_cells = by_tile[t]
            slabs = plan.tile_slabs[t]
            psum_t = ppool.tile([P, OUT], F32)
            root_t = rpool.tile([P, OUT], F32)
            nc.scalar.dma_start(root_t[:], rootp[t * P : (t + 1) * P, :])
            n_mm = 5 * len(slabs)
            for cell in tile_cells:
                fb = cell["fullB"]
                for g0 in range(0, fb, cfg.g_blk):
                    nb = min(cfg.g_blk, fb - g0)
                    n_mm += (nb // 2 + nb % 2) * R if cfg.double_row else nb * R
            done = 0
            for cell in tile_cells:
                fb, fg0, h = cell["fullB"], cell["fg0"], cell["h"]
                for g0 in range(0, fb, cfg.g_blk):
                    nb = min(cfg.g_blk, fb - g0)
                    gb = fg0 + g0
                    gt = gpool.tile([P, cfg.g_blk, ROW], F8, tag="g")
                    gather(gb * P, nb * P, h, gt[:, :nb, :])
                    lt = lpool.tile([P, cfg.g_blk, R, P], F8, tag="l")
                    nc.sync.dma_start(lt[:, :nb, :, :], lhsT[:, gb : gb + nb, :])
                    j = 0
                    while j < nb:
                        if cfg.double_row and j + 1 < nb:
                            for b in range(R):
                                nc.tensor.matmul(
                                    psum_t[:],
                                    lhsT=lt[:, j : j + 2, b, :],
                                    rhs=gt[:, j : j + 2, b * OUT : (b + 1) * OUT],
                                    start=(done == 0),
                                    stop=(done == n_mm - 1),
                                    perf_mode=mybir.MatmulPerfMode.DoubleRow,
                                )
                                done += 1
                            j += 2
                        else:
                            for b in range(R):
                                nc.tensor.matmul(
                                    psum_t[:],
                                    lhsT=lt[:, j, b, :],
                                    rhs=gt[:, j, b * OUT : (b + 1) * OUT],
                                    start=(done == 0),
                                    stop=(done == n_mm - 1),
                                )
                                done += 1
                            j += 1
            # tail slabs of this tile
            for (bi, jc, g) in slabs:
                tgt, tlt, g0 = live[bi]
                for b in range(R):
                    nc.tensor.matmul(
                        psum_t[:],
                        lhsT=tlt[:, g - g0, b, :],
                        rhs=tgt[:, jc, b * OUT : (b + 1) * OUT],
                        start=(done == 0),
                        stop=(done == n_mm - 1),
                    )
                    done += 1

            # epilogue: mean(+dequant) on ACT, +root(+bias), relu, log_softmax
            h_t = epool.tile([P, OUT], F32, tag="h")
            nc.scalar.activation(
                out=h_t[:], in_=psum_t[:],
                func=mybir.ActivationFunctionType.Copy,
                scale=rcnt_sb[:, t : t + 1],
            )
            nc.vector.tensor_tensor(out=h_t[:], in0=h_t[:], in1=root_t[:], op=add)
            if use_bias:
                nc.vector.tensor_tensor(
                    out=h_t[:], in0=h_t[:], in1=biast_sb[:], op=add
                )
            nc.vector.tensor_scalar_max(h_t[:], h_t[:], 0.0)
            mx = epool.tile([P, 2], F32, tag="mx")
            nc.vector.tensor_reduce(
                out=mx[:, 0:1], in_=h_t[:], axis=mybir.AxisListType.X,
                op=mybir.AluOpType.max, negate=True,
            )
            ex = epool.tile([P, OUT], F32, tag="ex")
            nc.scalar.activation(
                out=ex[:], in_=h_t[:], func=mybir.ActivationFunctionType.Exp,
                bias=mx[:, 0:1], scale=1.0, accum_out=mx[:, 1:2],
            )
            ln = epool.tile([P, 2], F32, tag="ln")
            nc.scalar.activation(
                out=ln[:, 0:1], in_=mx[:, 1:2], func=mybir.ActivationFunctionType.Ln,
            )
            tot = epool.tile([P, 2], F32, tag="tot")
            nc.vector.tensor_tensor(
                out=tot[:, 0:1], in0=mx[:, 0:1], in1=ln[:, 0:1],
                op=mybir.AluOpType.subtract,
            )
            o = epool.tile([P, OUT], F32, tag="o")
            nc.scalar.activation(
                out=o[:], in_=h_t[:],
                func=mybir.ActivationFunctionType.Identity,
                bias=tot[:, 0:1], scale=1.0,
            )
            nc.scalar.dma_start(out[t * P : (t + 1) * P, :], o[:])
    nc.compile()
    return nc


# ----------------------------------------------------------------------------
# Entry point
# ----------------------------------------------------------------------------

def _run(cfg: Cfg, inputs: dict, trace: bool = False):
    plan, in_maps, use_bias = host_prep(
        cfg,
        inputs["edge_index"], inputs["edge_type"], inputs["basis"],
        inputs["root"], inputs["bias"], inputs["comp"],
    )
    nc = build_program(cfg, plan, use_bias)
    res = run_bass_kernel_spmd(
        nc, in_maps, core_ids=list(range(cfg.n_cores)), trace=trace,
    )
    parts = [res.results[c]["out"] for c in range(cfg.n_cores)]
    full = np.concatenate(parts, axis=0)[: cfg.n_nodes]
    return np.ascontiguousarray(full.astype(np.float32)), res


def kernel(**inputs) -> np.ndarray:
    cfg = Cfg()
    out, _ = _run(cfg, inputs)
    return out


# revision 42
# speedup vs baseline: 1.0871x; 1.0258x over previous
"""RGCN (basis-decomposition, one-hot features) message passing on 8 trn2 NeuronCores.

Math (per reference):
    weight[r] = sum_b comp[r,b] * basis[b]          # [R, N, OUT]
    msg_e     = weight[edge_type_e, src_e]          # [E, OUT]
    agg       = segment_sum(msg, dst) / max(cnt, 1) # mean aggregation
    out       = log_softmax(relu(agg + root + bias), axis=1)

Distribution strategy: shard edges by DESTINATION node range across the 8
cores (each core owns N/8 output rows), so no collectives are needed.

Per 128-edge block the device:
  1. gathers, per edge, the 5 basis rows of the src node from an fp8e4
     table (basisQ[n] = 32 * basis[:, n, :], 1536 B contiguous per node)
     via gpsimd dma_gather,
  2. accumulates psum[tile] += lhsT_b.T @ basisrows_b for b = 0..4, where
     lhsT_b[e, nd] = comp[type_e, b] * (dst_e == nd) is a host-prepared
     comp-scaled one-hot slab (pure scatter of comp values driven by the
     edge structure; no arithmetic), streamed sequentially from DRAM in
     fp8e4.  This keeps DVE free: no on-device one-hot construction.

The Q7 software descriptor generation of dma_gather has a large fixed
cost per call, so per-cell remainder blocks ("tails", <=128 rows) are
packed 16-aligned into per-half batches of 8 tails and fetched with ONE
gather call per batch, prefetched one 8-tile window ahead of their first
consumer; a tail that lands across a 128-slot column boundary simply
gets two tile-restricted lhsT slabs.  The epilogue (mean incl. the 1/32
dequant, +root, +bias, relu, log_softmax) runs fused per tile on
DVE/ACT and streams the rows out.

Host-side work is limited to index metadata (sharding, grouping edges by
destination tile, padding, bincount, scatter of comp values into one-hot
positions) and data layout (transpose/pad/quantize of input tensors).
All floating-point arithmetic on tensor data runs on device.
"""

import math
from contextlib import ExitStack
from dataclasses import dataclass

import numpy as np

import ml_dtypes

import concourse.bacc as bacc
import concourse.bass as bass
import concourse.mybir as mybir
import concourse.tile as tile
from concourse.bass_utils import run_bass_kernel_spmd

F32 = mybir.dt.float32
F8 = mybir.dt.float8e4
I16 = mybir.dt.int16
NP_F8 = ml_dtypes.float8_e4m3
P = 128  # SBUF partitions


@dataclass
class Cfg:
    n_nodes: int = 50000
    n_rel: int = 5
    n_bases: int = 5
    out_dim: int = 300
    n_cores: int = 8
    g_blk: int = 8          # gather-group size in 128-edge blocks (<=8: 1024 idx/call)
    tb_sz: int = 8          # tails per batched tail gather call
    scale: float = 32.0     # power-of-2 exponent shift for the fp8 basis table
    double_row: bool = True   # fp8 DoubleRow matmuls (2 edge blocks / matmul)
    swdge_queues: int = 4   # SWDGE queues to round-robin dma_gather over
    act_preload: bool = True  # preload the Exp+Ln act table set once

    @property
    def rowp(self) -> int:  # elements per fp8 basisQ row (256B multiple)
        return (self.row + 255) // 256 * 256

    @property
    def half(self) -> int:  # src-range half size for int16 gather indices
        return (self.n_nodes + 1) // 2

    @property
    def row(self) -> int:  # payload elements per basisQ row
        return self.n_bases * self.out_dim

    @property
    def lrow(self) -> int:  # elements per lhsT slab row (5 one-hot rows)
        return self.n_rel * P

    @property
    def npc(self) -> int:  # nodes per core, multiple of 128
        return ((self.n_nodes + self.n_cores - 1) // self.n_cores + P - 1) // P * P

    @property
    def n_tiles(self) -> int:
        return self.npc // P


def _c16(x):
    return (x + 15) // 16 * 16


class Plan:
    """Shared (cross-core) slot/slab layout derived from max counts."""

    def __init__(self, cfg: Cfg, counts):
        T = cfg.n_tiles
        self.cells = []  # dicts
        fgid = 0
        for t in range(T):
            for h in range(2):
                mx = int(counts[:, t, h].max())
                if mx == 0 and h == 1 and counts[:, t, 0].max() == 0:
                    mx = 16  # dummy tail keeps the tile's psum zeroed
                elif mx == 0:
                    continue
                fullB = (mx - 1) // P
                self.cells.append(dict(
                    t=t, h=h, fullB=fullB, mx=mx, fg0=fgid,
                    tlen=_c16(mx - fullB * P), toff=-1, colgid={},
                ))
                fgid += fullB
        self.NBF = fgid

        # tail batches per half, tiles in order
        tails = {0: [c for c in self.cells if c["h"] == 0],
                 1: [c for c in self.cells if c["h"] == 1]}
        self.batches = []
        soff = self.NBF * P
        gid = self.NBF
        kmax = max((len(v) + cfg.tb_sz - 1) // cfg.tb_sz for v in tails.values())
        for k in range(kmax):
            for h in (0, 1):
                grp = tails[h][k * cfg.tb_sz : (k + 1) * cfg.tb_sz]
                if not grp:
                    continue
                off = 0
                slabs = []
                g0 = gid
                for cell in grp:
                    cell["toff"] = soff + off
                    for jc in range(off // P, (off + cell["tlen"] - 1) // P + 1):
                        cell["colgid"][jc] = gid
                        slabs.append((cell["t"], jc, gid))
                        gid += 1
                    off += cell["tlen"]
                span = (off + P - 1) // P * P
                self.batches.append(dict(
                    h=h, soff=soff, nidx=span, g0=g0, slabs=slabs,
                    last_t=grp[-1]["t"], first_t=grp[0]["t"],
                ))
                soff += span
        self.NS = soff
        self.NSLAB = gid
        self.tile_slabs = [[] for _ in range(T)]
        for bi, bt in enumerate(self.batches):
            for (t, jc, g) in bt["slabs"]:
                self.tile_slabs[t].append((bi, jc, g))
        # prefetch: issue each batch one tb_sz-tile window BEFORE its first
        # consumer so its gather never sits on the critical chain
        self.issue_at = [[] for _ in range(T)]
        for bi, bt in enumerate(self.batches):
            self.issue_at[max(0, bt["first_t"] - cfg.tb_sz)].append(bi)
        self.max_slabs = max(len(bt["slabs"]) for bt in self.batches)


# ----------------------------------------------------------------------------
# Host-side prep: index metadata + layout only (no float math on tensor data)
# ----------------------------------------------------------------------------

def host_prep(cfg: Cfg, edge_index, edge_type, basis, root, bias, comp):
    src = np.asarray(edge_index[0]).astype(np.int64)
    dst = np.asarray(edge_index[1]).astype(np.int64)
    etype = np.asarray(edge_type).astype(np.int64)
    basis = np.asarray(basis, dtype=np.float32)
    root = np.asarray(root, dtype=np.float32)
    bias = np.asarray(bias, dtype=np.float32)
    comp = np.asarray(comp, dtype=np.float32)

    N, R, B, OUT = cfg.n_nodes, cfg.n_rel, cfg.n_bases, cfg.out_dim
    NPC, T = cfg.npc, cfg.n_tiles
    HALF = cfg.half

    basisQ = np.zeros((N, cfg.rowp), dtype=NP_F8)
    basisQ[:, : B * OUT] = (
        basis.transpose(1, 0, 2).reshape(N, B * OUT) * cfg.scale
    ).astype(NP_F8)
    comp_f8 = comp.astype(NP_F8)
    biast = np.ascontiguousarray(np.broadcast_to(bias[None, :], (P, OUT)))

    core_of = dst // NPC
    per_core = []
    counts = np.zeros((cfg.n_cores, T, 2), dtype=np.int64)
    for c in range(cfg.n_cores):
        m = core_of == c
        s_c, t_c, dl_c = src[m], etype[m], dst[m] - c * NPC
        tid = dl_c // P
        hid = s_c // HALF
        order = np.argsort(tid * 2 + hid, kind="stable")
        s_c, t_c, dl_c = s_c[order], t_c[order], dl_c[order]
        tid, hid = tid[order], hid[order]
        for t in range(T):
            for h in range(2):
                counts[c, t, h] = np.count_nonzero((tid == t) & (hid == h))
        per_core.append((s_c, t_c, dl_c))

    plan = Plan(cfg, counts)
    NS, NSLAB = plan.NS, plan.NSLAB

    in_maps = []
    for c in range(cfg.n_cores):
        s_c, t_c, dl_c = per_core[c]
        srcs = np.zeros((NS,), dtype=np.int64)
        typs = np.zeros((NS,), dtype=np.int64)
        dloc = np.zeros((NS,), dtype=np.int64)
        gids = np.zeros((NS,), dtype=np.int64)
        valid = np.zeros((NS,), dtype=bool)
        pos = np.concatenate([[0], np.cumsum(counts[c].reshape(-1))])
        for cell in plan.cells:
            t, h, fullB = cell["t"], cell["h"], cell["fullB"]
            gi = t * 2 + h
            a, b = pos[gi], pos[gi + 1]
            n = b - a
            if n == 0:
                continue
            q = min(n, fullB * P)
            fs = cell["fg0"] * P
            srcs[fs : fs + q] = s_c[a : a + q] - h * HALF
            typs[fs : fs + q] = t_c[a : a + q]
            dloc[fs : fs + q] = dl_c[a : a + q] - t * P
            gids[fs : fs + q] = cell["fg0"] + np.arange(q) // P
            valid[fs : fs + q] = True
            nt = n - q
            if nt > 0:
                ts0 = cell["toff"]
                abs_sl = ts0 + np.arange(nt)
                srcs[abs_sl] = s_c[a + q : b] - h * HALF
                typs[abs_sl] = t_c[a + q : b]
                dloc[abs_sl] = dl_c[a + q : b] - t * P
                bt = plan.batches[[i for i, x in enumerate(plan.batches)
                                   if x["soff"] <= ts0 < x["soff"] + x["nidx"]][0]]
                cols = (abs_sl - bt["soff"]) // P
                cg = cell["colgid"]
                gids[abs_sl] = np.array([cg[int(j)] for j in cols])
                valid[abs_sl] = True

        # comp-scaled one-hot lhsT slabs (pure scatter of comp values)
        lhsT = np.zeros((P, NSLAB, R, P), dtype=NP_F8)
        pv = np.arange(NS) % P
        lhsT[pv[valid], gids[valid], :, dloc[valid]] = comp_f8[typs[valid], :]

        idx16 = np.ascontiguousarray(
            np.tile(srcs.astype(np.int16).reshape(NS // 16, 16).T, (8, 1))
        )  # [128, NS/16] int16
        idx_cols = idx16.view(np.float32)  # [128, NS/32]

        cnt = np.zeros((NPC,), dtype=np.float32)
        np.add.at(cnt, dl_c, 1.0)
        cnt2 = cnt.reshape(T, P).T.copy()

        rootp = np.zeros((NPC, OUT), dtype=np.float32)
        lo, hi = c * NPC, min((c + 1) * NPC, N)
        if hi > lo:
            rootp[: hi - lo] = root[lo:hi]

        consts = np.concatenate([idx_cols, cnt2, biast], axis=1).copy()
        in_maps.append(dict(basisQ=basisQ, consts=consts, lhsT=lhsT, rootp=rootp))
    return plan, in_maps, bool(np.any(bias != 0.0))


# ----------------------------------------------------------------------------
# Device program
# ----------------------------------------------------------------------------

def build_program(cfg: Cfg, plan: Plan, use_bias: bool):
    N, R, OUT, ROW = cfg.n_nodes, cfg.n_rel, cfg.out_dim, cfg.rowp
    NPC, T = cfg.npc, cfg.n_tiles
    HALF = cfg.half
    LROW = cfg.lrow
    NS, NSLAB = plan.NS, plan.NSLAB
    IDXW = NS // 32  # f32 cols holding int16 gather indices

    nc = bacc.Bacc(
        "TRN2",
        target_bir_lowering=False,
        debug=False,
        enable_asserts=False,
        num_devices=cfg.n_cores,
        num_swdge_queues=cfg.swdge_queues,
    )
    W = IDXW + T + OUT
    basisQ = nc.dram_tensor("basisQ", [N, ROW], F8, kind="ExternalInput").ap()
    lhsT = nc.dram_tensor("lhsT", [P, NSLAB, LROW], F8, kind="ExternalInput").ap()
    consts = nc.dram_tensor("consts", [P, W], F32, kind="ExternalInput").ap()
    rootp = nc.dram_tensor("rootp", [NPC, OUT], F32, kind="ExternalInput").ap()
    out = nc.dram_tensor("out", [NPC, OUT], F32, kind="ExternalOutput").ap()

    add = mybir.AluOpType.add

    by_tile = [[] for _ in range(T)]
    for cell in plan.cells:
        by_tile[cell["t"]].append(cell)

    with tile.TileContext(nc) as tc, ExitStack() as ctx:
        cpool = ctx.enter_context(tc.tile_pool(name="const", bufs=1))
        gpool = ctx.enter_context(tc.tile_pool(name="g", bufs=6))
        lpool = ctx.enter_context(tc.tile_pool(name="lhs", bufs=4))
        tgpool = ctx.enter_context(tc.tile_pool(name="tg", bufs=4))
        tlpool = ctx.enter_context(tc.tile_pool(name="tl", bufs=4))
        ppool = ctx.enter_context(tc.tile_pool(name="ps", bufs=4, space="PSUM"))
        rpool = ctx.enter_context(tc.tile_pool(name="root", bufs=3))
        epool = ctx.enter_context(tc.tile_pool(name="epi", bufs=3))

        if cfg.act_preload:
            from concourse.hw_specs import get_activation_tables

            A = mybir.ActivationFunctionType
            tabs = list(get_activation_tables(nc.m.arch).values())
            set_id = next(i for i, s in enumerate(tabs) if A.Exp in s and A.Ln in s)
            pre = mybir.InstLoadActFuncSet(
                name=nc.get_next_instruction_name(),
                ins=[], outs=[], act_func_set_id=set_id,
            )
            pre.engine = mybir.EngineType.Activation
            nc.add_instruction(pre)

        consts_sb = cpool.tile([P, W], F32)
        c0 = min(8 * cfg.g_blk, IDXW)
        nc.scalar.dma_start(consts_sb[:, 0:c0], consts[:, 0:c0])
        nc.scalar.dma_start(consts_sb[:, c0:IDXW], consts[:, c0:IDXW])
        nc.scalar.dma_start(consts_sb[:, IDXW:W], consts[:, IDXW:W])
        idx16_all = consts_sb[:, 0:IDXW].bitcast(I16)  # [128, NS/16]
        cnt_sb = consts_sb[:, IDXW : IDXW + T]
        biast_sb = consts_sb[:, IDXW + T : IDXW + T + OUT]
        rcnt_sb = cpool.tile([P, T], F32)
        nc.vector.tensor_scalar_max(rcnt_sb[:], cnt_sb, 1.0)
        nc.vector.reciprocal(rcnt_sb[:], rcnt_sb[:])
        nc.vector.tensor_scalar(
            out=rcnt_sb[:], in0=rcnt_sb[:], scalar1=1.0 / cfg.scale,
            scalar2=None, op0=mybir.AluOpType.mult,
        )

        def gather(soff, nidx, h, out_ap):
            nonlocal gq
            hi = min((h + 1) * HALF, N)
            nc.gpsimd.dma_gather(
                out_ap=out_ap,
                in_ap=basisQ[h * HALF : hi, :],
                idxs_ap=idx16_all[:, soff // 16 : (soff + nidx) // 16],
                num_idxs=nidx,
                num_idxs_reg=nidx,
                elem_size=ROW,
                queue_num=gq,
            )
            gq = (gq + 1) % cfg.swdge_queues

        gq = 0
        lq = 0
        live = {}  # batch idx -> (tgt, tlt, g0)
        for t in range(T):
            # prefetch tail batches one window ahead of their consumers
            for bi in plan.issue_at[t]:
                bt = plan.batches[bi]
                tgt = tgpool.tile([P, cfg.tb_sz, ROW], F8, tag="tg")
                nj = bt["nidx"] // P
                gather(bt["soff"], bt["nidx"], bt["h"], tgt[:, :nj, :])
                tlt = tlpool.tile([P, plan.max_slabs, R, P], F8, tag="tl")
                ns = len(bt["slabs"])
                nc.scalar.dma_start(
                    tlt[:, :ns, :, :], lhsT[:, bt["g0"] : bt["g0"] + ns, :]
                )
                live[bi] = (tgt, tlt, bt["g0"])

            tile_cells = by_tile[t]
            slabs = plan.tile_slabs[t]
            psum_t = ppool.tile([P, OUT], F32)
            root_t = rpool.tile([P, OUT], F32)
            nc.scalar.dma_start(root_t[:], rootp[t * P : (t + 1) * P, :])
            n_mm = 5 * len(slabs)
            for cell in tile_cells:
                fb = cell["fullB"]
                for g0 in range(0, fb, cfg.g_blk):
                    nb = min(cfg.g_blk, fb - g0)
                    n_mm += (nb // 2 + nb % 2) * R if cfg.double_row else nb * R
            done = 0
            for cell in tile_cells:
                fb, fg0, h = cell["fullB"], cell["fg0"], cell["h"]
                for g0 in range(0, fb, cfg.g_blk):
                    nb = min(cfg.g_blk, fb - g0)
                    gb = fg0 + g0
                    gt = gpool.tile([P, cfg.g_blk, ROW], F8, tag="g")
                    gather(gb * P, nb * P, h, gt[:, :nb, :])
                    lt = lpool.tile([P, cfg.g_blk, R, P], F8, tag="l")
                    nc.sync.dma_start(lt[:, :nb, :, :], lhsT[:, gb : gb + nb, :])
                    j = 0
                    while j < nb:
                        if cfg.double_row and j + 1 < nb:
                            for b in range(R):
                                nc.tensor.matmul(
                                    psum_t[:],
                                    lhsT=lt[:, j : j + 2, b, :],
                                    rhs=gt[:, j : j + 2, b * OUT : (b + 1) * OUT],
                                    start=(done == 0),
                                    stop=(done == n_mm - 1),
                                    perf_mode=mybir.MatmulPerfMode.DoubleRow,
                                )
                                done += 1
                            j += 2
                        else:
                            for b in range(R):
                                nc.tensor.matmul(
                                    psum_t[:],
                                    lhsT=lt[:, j, b, :],
                                    rhs=gt[:, j, b * OUT : (b + 1) * OUT],
                                    start=(done == 0),
                                    stop=(done == n_mm - 1),
                                )
                                done += 1
                            j += 1
            # tail slabs of this tile
            for (bi, jc, g) in slabs:
                tgt, tlt, g0 = live[bi]
                for b in range(R):
                    nc.tensor.matmul(
                        psum_t[:],
                        lhsT=tlt[:, g - g0, b, :],
                        rhs=tgt[:, jc, b * OUT : (b + 1) * OUT],
                        start=(done == 0),
                        stop=(done == n_mm - 1),
                    )
                    done += 1

            # epilogue: mean(+dequant) on ACT, +root(+bias), relu, log_softmax
            h_t = epool.tile([P, OUT], F32, tag="h")
            nc.scalar.activation(
                out=h_t[:], in_=psum_t[:],
                func=mybir.ActivationFunctionType.Copy,
                scale=rcnt_sb[:, t : t + 1],
            )
            nc.vector.tensor_tensor(out=h_t[:], in0=h_t[:], in1=root_t[:], op=add)
            if use_bias:
                nc.vector.tensor_tensor(
                    out=h_t[:], in0=h_t[:], in1=biast_sb[:], op=add
                )
            nc.vector.tensor_scalar_max(h_t[:], h_t[:], 0.0)
            mx = epool.tile([P, 2], F32, tag="mx")
            nc.vector.tensor_reduce(
                out=mx[:, 0:1], in_=h_t[:], axis=mybir.AxisListType.X,
                op=mybir.AluOpType.max, negate=True,
            )
            ex = epool.tile([P, OUT], F32, tag="ex")
            nc.scalar.activation(
                out=ex[:], in_=h_t[:], func=mybir.ActivationFunctionType.Exp,
                bias=mx[:, 0:1], scale=1.0, accum_out=mx[:, 1:2],
            )
            ln = epool.tile([P, 2], F32, tag="ln")
            nc.scalar.activation(
                out=ln[:, 0:1], in_=mx[:, 1:2], func=mybir.ActivationFunctionType.Ln,
            )
            tot = epool.tile([P, 2], F32, tag="tot")
            nc.vector.tensor_tensor(
                out=tot[:, 0:1], in0=mx[:, 0:1], in1=ln[:, 0:1],
                op=mybir.AluOpType.subtract,
            )
            o = epool.tile([P, OUT], F32, tag="o")
            nc.scalar.activation(
                out=o[:], in_=h_t[:],
                func=mybir.ActivationFunctionType.Identity,
                bias=tot[:, 0:1], scale=1.0,
            )
            nc.scalar.dma_start(out[t * P : (t + 1) * P, :], o[:])
    nc.compile()
    return nc


# ----------------------------------------------------------------------------
# Entry point
# ----------------------------------------------------------------------------

def _run(cfg: Cfg, inputs: dict, trace: bool = False):
    plan, in_maps, use_bias = host_prep(
        cfg,
        inputs["edge_index"], inputs["edge_type"], inputs["basis"],
        inputs["root"], inputs["bias"], inputs["comp"],
    )
    nc = build_program(cfg, plan, use_bias)
    res = run_bass_kernel_spmd(
        nc, in_maps, core_ids=list(range(cfg.n_cores)), trace=trace,
    )
    parts = [res.results[c]["out"] for c in range(cfg.n_cores)]
    full = np.concatenate(parts, axis=0)[: cfg.n_nodes]
    return np.ascontiguousarray(full.astype(np.float32)), res


def kernel(**inputs) -> np.ndarray:
    cfg = Cfg()
    out, _ = _run(cfg, inputs)
    return out
